# revision 6
# baseline (speedup 1.0000x reference)
"""MIL gated-attention pooling kernel for Trainium2 (8 NeuronCores, SPMD).

Problem (per reference):
    A_pre = tanh(x@W1 + b1) * sigmoid(x@W3 + b3)      # [N, H]
    A     = A_pre @ W2 + b2                           # [N, K]
    P     = softmax over instances per (bag, head)    # [B, K, L]
    out   = einsum('bkl,bld->bkd', P, x) -> [B, K*D]

Shapes hardcoded: B=32 bags, L=2048 instances/bag, D=512, H=256, K=4.
Sharding: data-parallel over bags, 4 bags (8192 rows) per core, weights
replicated. No cross-core communication.

Device algorithm per core (all matmuls bf16 inputs, fp32 PSUM accum):
  - logits path consumes a host-pretransposed xT [D, 8192] so the
    contraction dim (d) sits on SBUF partitions for the PE.
  - sigmoid(h) = 0.5*(1+tanh(h/2)); the 0.5 is folded into W2 on the
    host, so ACT only ever needs tanh+exp (one table set, no thrash).
  - b2 and the softmax max-subtraction are dropped: both cancel in
    softmax (b2 is constant along the softmax axis; logits are O(1)).
  - softmax denominator Z comes from a ones-column appended to x on the
    host, accumulated by the same PE weights as the weighted sum.
"""

import numpy as np
import ml_dtypes
from contextlib import ExitStack

B, L, D, H, K = 32, 2048, 512, 256, 4
NCORES = 8
BPC = B // NCORES       # bags per core = 4
R = BPC * L             # rows per core = 8192
NTILE = 512             # instance-tile (matmul free dim)
NT = R // NTILE         # n-tiles per core = 16
NTB = L // NTILE        # n-tiles per bag = 4
NCH = L // 128          # 128-row chunks per bag = 16
DC = D // 128           # contraction chunks = 4
HC = 2 * H // 128       # output-channel chunks of [W1|W3] = 4

_BF16 = ml_dtypes.bfloat16
_CACHE = {}


def _build_nc():
    import concourse.bacc as bacc
    import concourse.tile as tile
    import concourse.mybir as mybir

    dt = mybir.dt
    AF = mybir.ActivationFunctionType

    nc = bacc.Bacc("TRN2", target_bir_lowering=False, debug=False)
    xT = nc.dram_tensor("xT", [D, R], dt.bfloat16, kind="ExternalInput").ap()
    xa = nc.dram_tensor("xa", [R, 513], dt.bfloat16, kind="ExternalInput").ap()
    w13 = nc.dram_tensor("w13", [128, DC, 2 * H], dt.bfloat16, kind="ExternalInput").ap()
    w2 = nc.dram_tensor("w2", [128, 2, K], dt.bfloat16, kind="ExternalInput").ap()
    b13 = nc.dram_tensor("b13", [128, HC], dt.float32, kind="ExternalInput").ap()
    out = nc.dram_tensor("out", [BPC, K, D], dt.float32, kind="ExternalOutput").ap()

    # transposed-DMA views: xT as [p, dc, n]; xa as [p, chunk, col]
    xT_v = xT.rearrange("(dc p) n -> p dc n", p=128)
    xa_v = xa.rearrange("(g p) f -> p g f", p=128)  # g = global 128-row chunk

    with tile.TileContext(nc) as tc, ExitStack() as ctx:
        consts = ctx.enter_context(tc.tile_pool(name="consts", bufs=1))
        xtp = ctx.enter_context(tc.tile_pool(name="xtp", bufs=4))
        tsp = ctx.enter_context(tc.tile_pool(name="tsp", bufs=8))
        app = ctx.enter_context(tc.tile_pool(name="app", bufs=6))
        epool = ctx.enter_context(tc.tile_pool(name="epool", bufs=2))
        xap = ctx.enter_context(tc.tile_pool(name="xap", bufs=5))
        opool = ctx.enter_context(tc.tile_pool(name="opool", bufs=2))
        rpool = ctx.enter_context(tc.tile_pool(name="rpool", bufs=2))
        import concourse.bass as bass

        psH = ctx.enter_context(tc.tile_pool(name="psH", bufs=4, space=bass.MemorySpace.PSUM))
        psA = ctx.enter_context(tc.tile_pool(name="psA", bufs=2, space=bass.MemorySpace.PSUM))
        psU = ctx.enter_context(tc.tile_pool(name="psU", bufs=1, space=bass.MemorySpace.PSUM))
        psZ = ctx.enter_context(tc.tile_pool(name="psZ", bufs=1, space=bass.MemorySpace.PSUM))

        w13_sb = consts.tile([128, DC, 2 * H], dt.bfloat16)
        nc.sync.dma_start(out=w13_sb[:], in_=w13[:])
        w2_sb = consts.tile([128, 2, K], dt.bfloat16)
        nc.sync.dma_start(out=w2_sb[:], in_=w2[:])
        b13_sb = consts.tile([128, HC], dt.float32)
        nc.sync.dma_start(out=b13_sb[:], in_=b13[:])

        def logits_ntile(bag, ntl, a_ps):
            """One 512-instance tile of the gated-attention logit path."""
            if True:
                n0 = (bag * NTB + ntl) * NTILE
                xt = xtp.tile([128, DC, NTILE], dt.bfloat16, tag="xt")
                nc.sync.dma_start(out=xt[:], in_=xT_v[:, :, n0:n0 + NTILE])
                ap_tiles = []
                for pair in range(2):  # h-chunks: tanh-branch, sigmoid-branch
                    h1 = psH.tile([128, NTILE], dt.float32, tag="h")
                    for dc in range(DC):
                        nc.tensor.matmul(
                            h1[:],
                            w13_sb[:, dc, pair * 128:(pair + 1) * 128],
                            xt[:, dc, :],
                            start=(dc == 0),
                            stop=(dc == DC - 1),
                        )
                    t = tsp.tile([128, NTILE], dt.bfloat16, tag="ts")
                    nc.scalar.activation(
                        t[:], h1[:], AF.Tanh, bias=b13_sb[:, pair:pair + 1], scale=1.0
                    )
                    h3 = psH.tile([128, NTILE], dt.float32, tag="h")
                    for dc in range(DC):
                        nc.tensor.matmul(
                            h3[:],
                            w13_sb[:, dc, (pair + 2) * 128:(pair + 3) * 128],
                            xt[:, dc, :],
                            start=(dc == 0),
                            stop=(dc == DC - 1),
                        )
                    s = tsp.tile([128, NTILE], dt.bfloat16, tag="ts")
                    nc.scalar.activation(
                        s[:], h3[:], AF.Tanh, bias=b13_sb[:, pair + 2:pair + 3], scale=0.5
                    )
                    # gated = t*sigmoid = 0.5*(t*s + t); the 0.5 lives in w2
                    apt = app.tile([128, NTILE], dt.bfloat16, tag="ap")
                    nc.vector.tensor_mul(out=apt[:], in0=t[:], in1=s[:])
                    nc.vector.tensor_add(out=apt[:], in0=apt[:], in1=t[:])
                    ap_tiles.append(apt)
                for sub in range(NTILE // 128):
                    c = ntl * (NTILE // 128) + sub
                    for pair in range(2):
                        nc.tensor.matmul(
                            a_ps[:, K * c:K * (c + 1)],
                            ap_tiles[pair][:, sub * 128:(sub + 1) * 128],
                            w2_sb[:, pair, :],
                            start=(pair == 0),
                            stop=(pair == 1),
                        )
        def wsum_group(bag, grp, e_sb, u_ps, z_ps):
            """4 x-chunks of one bag's exp-weighted sum + normalizer."""
            g0 = bag * NCH + grp * 4
            xat = xap.tile([128, 4, 513], dt.bfloat16, tag="xa")
            nc.sync.dma_start(out=xat[:], in_=xa_v[:, g0:g0 + 4, :])
            for j in range(4):
                c = grp * 4 + j
                lhs = e_sb[:, K * c:K * (c + 1)]
                nc.tensor.matmul(
                    u_ps[:], lhs, xat[:, j, 0:D],
                    start=(c == 0), stop=(c == NCH - 1),
                )
                nc.tensor.matmul(
                    z_ps[:], lhs, xat[:, j, D:513],
                    start=(c == 0), stop=(c == NCH - 1),
                )

        def wsum_finish(bag, u_ps, z_ps):
            r_sb = rpool.tile([K, 1], dt.float32, tag="r")
            nc.vector.reciprocal(out=r_sb[:], in_=z_ps[:])
            o_sb = opool.tile([K, D], dt.float32, tag="o")
            nc.vector.tensor_scalar_mul(out=o_sb[:], in0=u_ps[:], scalar1=r_sb[:])
            nc.sync.dma_start(out=out[bag], in_=o_sb[:])

        # Software pipeline, one-bag skew at n-tile granularity: bag b's four
        # weighted-sum groups are interleaved between bag b+1's logits tiles,
        # so the PE always has dense matmul work ahead of any x-chunk DMA
        # wait and the HAM clock stays warm until the final group.
        prev = None  # (bag, e_sb, u_ps, z_ps)
        for bag in range(BPC):
            a_ps = psA.tile([128, K * NCH], dt.float32, tag="a")
            for ntl in range(NTB):
                logits_ntile(bag, ntl, a_ps)
                if prev is not None:
                    pb, pe_sb, pu, pz = prev
                    wsum_group(pb, ntl, pe_sb, pu, pz)
            if prev is not None:
                pb, pe_sb, pu, pz = prev
                wsum_finish(pb, pu, pz)
            e_sb = epool.tile([128, K * NCH], dt.bfloat16, tag="e")
            nc.scalar.activation(e_sb[:], a_ps[:], AF.Exp)
            u_ps = psU.tile([K, D], dt.float32, tag="u")
            z_ps = psZ.tile([K, 1], dt.float32, tag="z")
            prev = (bag, e_sb, u_ps, z_ps)
        pb, pe_sb, pu, pz = prev
        for grp in range(NCH // 4):
            wsum_group(pb, grp, pe_sb, pu, pz)
        wsum_finish(pb, pu, pz)

    nc.compile()
    return nc


def get_nc():
    if "nc" not in _CACHE:
        _CACHE["nc"] = _build_nc()
    return _CACHE["nc"]


def make_in_maps(x, W1, b1, W3, b3, W2, b2):
    x = np.asarray(x, dtype=np.float32)
    W1 = np.asarray(W1, dtype=np.float32)
    W3 = np.asarray(W3, dtype=np.float32)
    W2 = np.asarray(W2, dtype=np.float32)
    b1 = np.asarray(b1, dtype=np.float32)
    b3 = np.asarray(b3, dtype=np.float32)

    # [W1 | W3] with lhsT layout [p, dc, h']: element = W13[dc*128+p, h']
    w13 = np.concatenate([W1, W3], axis=1)          # [512, 512]
    w13_t = np.ascontiguousarray(
        w13.reshape(DC, 128, 2 * H).transpose(1, 0, 2)
    ).astype(_BF16)
    # 0.5 * W2 with layout [p, hc, k]
    w2_t = np.ascontiguousarray(
        (0.5 * W2).reshape(2, 128, K).transpose(1, 0, 2)
    ).astype(_BF16)
    # biases [p, j]: j in {0,1} -> b1 chunks, {2,3} -> 0.5*b3 chunks
    b13 = np.concatenate([b1, 0.5 * b3]).reshape(HC, 128).T
    b13 = np.ascontiguousarray(b13, dtype=np.float32)

    in_maps = []
    for c in range(NCORES):
        xc = x[c * R:(c + 1) * R]                   # [8192, 512] fp32
        xT_np = np.ascontiguousarray(xc.T).astype(_BF16)
        xa_np = np.empty((R, 513), dtype=_BF16)
        xa_np[:, :D] = xc.astype(_BF16)
        xa_np[:, D] = _BF16(1.0)
        in_maps.append(
            {"xT": xT_np, "xa": xa_np, "w13": w13_t, "w2": w2_t, "b13": b13}
        )
    return in_maps


def kernel(x, W1, b1, W3, b3, W2, b2, bag_lengths):
    from concourse.bass_utils import run_bass_kernel_spmd

    nc = get_nc()
    in_maps = make_in_maps(x, W1, b1, W3, b3, W2, b2)
    res = run_bass_kernel_spmd(nc, in_maps, list(range(NCORES)))
    out = np.empty((B, K * D), dtype=np.float32)
    for c in range(NCORES):
        out[c * BPC:(c + 1) * BPC] = res.results[c]["out"].reshape(BPC, K * D)
    return out


# revision 10
# speedup vs baseline: 1.0088x; 1.0088x over previous
"""MIL gated-attention pooling kernel for Trainium2 (8 NeuronCores, SPMD).

Problem (per reference):
    A_pre = tanh(x@W1 + b1) * sigmoid(x@W3 + b3)      # [N, H]
    A     = A_pre @ W2 + b2                           # [N, K]
    P     = softmax over instances per (bag, head)    # [B, K, L]
    out   = einsum('bkl,bld->bkd', P, x) -> [B, K*D]

Shapes hardcoded: B=32 bags, L=2048 instances/bag, D=512, H=256, K=4.
Sharding: data-parallel over bags, 4 bags (8192 rows) per core, weights
replicated. No cross-core communication.

Device algorithm per core (all matmuls bf16 inputs, fp32 PSUM accum):
  - logits path consumes a host-pretransposed xT [D, 8192] so the
    contraction dim (d) sits on SBUF partitions for the PE.
  - sigmoid(h) = 0.5*(1+tanh(h/2)); the 0.5 is folded into W2 on the
    host, so ACT only ever needs tanh+exp (one table set, no thrash).
  - b2 and the softmax max-subtraction are dropped: both cancel in
    softmax (b2 is constant along the softmax axis; logits are O(1)).
  - softmax denominator Z comes from a ones-column appended to x on the
    host, accumulated by the same PE weights as the weighted sum.
"""

import numpy as np
import ml_dtypes
from contextlib import ExitStack

B, L, D, H, K = 32, 2048, 512, 256, 4
NCORES = 8
BPC = B // NCORES       # bags per core = 4
R = BPC * L             # rows per core = 8192
NTILE = 512             # instance-tile (matmul free dim)
NT = R // NTILE         # n-tiles per core = 16
NTB = L // NTILE        # n-tiles per bag = 4
NCH = L // 128          # 128-row chunks per bag = 16
DC = D // 128           # contraction chunks = 4
HC = 2 * H // 128       # output-channel chunks of [W1|W3] = 4

_BF16 = ml_dtypes.bfloat16
_CACHE = {}


def _build_nc():
    import concourse.bacc as bacc
    import concourse.tile as tile
    import concourse.mybir as mybir

    dt = mybir.dt
    AF = mybir.ActivationFunctionType

    nc = bacc.Bacc("TRN2", target_bir_lowering=False, debug=False)
    xT = nc.dram_tensor("xT", [D, R], dt.bfloat16, kind="ExternalInput").ap()
    xa = nc.dram_tensor("xa", [R, 513], dt.bfloat16, kind="ExternalInput").ap()
    w13 = nc.dram_tensor("w13", [128, DC, 2 * H], dt.bfloat16, kind="ExternalInput").ap()
    w2 = nc.dram_tensor("w2", [128, 2, K], dt.bfloat16, kind="ExternalInput").ap()
    b13 = nc.dram_tensor("b13", [128, HC], dt.float32, kind="ExternalInput").ap()
    out = nc.dram_tensor("out", [BPC, K, D], dt.float32, kind="ExternalOutput").ap()

    # transposed-DMA views: xT as [p, dc, n]; xa as [p, chunk, col]
    xT_v = xT.rearrange("(dc p) n -> p dc n", p=128)
    xa_v = xa.rearrange("(g p) f -> p g f", p=128)  # g = global 128-row chunk

    with tile.TileContext(nc) as tc, ExitStack() as ctx:
        consts = ctx.enter_context(tc.tile_pool(name="consts", bufs=1))
        xtp = ctx.enter_context(tc.tile_pool(name="xtp", bufs=4))
        tsp = ctx.enter_context(tc.tile_pool(name="tsp", bufs=8))
        app = ctx.enter_context(tc.tile_pool(name="app", bufs=6))
        epool = ctx.enter_context(tc.tile_pool(name="epool", bufs=4))
        xap = ctx.enter_context(tc.tile_pool(name="xap", bufs=5))
        opool = ctx.enter_context(tc.tile_pool(name="opool", bufs=2))
        rpool = ctx.enter_context(tc.tile_pool(name="rpool", bufs=2))
        import concourse.bass as bass

        psH = ctx.enter_context(tc.tile_pool(name="psH", bufs=4, space=bass.MemorySpace.PSUM))
        psA = ctx.enter_context(tc.tile_pool(name="psA", bufs=1, space=bass.MemorySpace.PSUM))
        psU = ctx.enter_context(tc.tile_pool(name="psU", bufs=2, space=bass.MemorySpace.PSUM))
        psZ = ctx.enter_context(tc.tile_pool(name="psZ", bufs=1, space=bass.MemorySpace.PSUM))

        # constants: w13 split per-dc so the first [128,128] weight block and
        # the first rhs chunk land ASAP after the DMA path warms up.
        w13_sb = consts.tile([128, DC, 2 * H], dt.bfloat16)
        nc.sync.dma_start(out=w13_sb[:, 0, :], in_=w13[:, 0, :])
        w2_sb = consts.tile([128, 2, K], dt.bfloat16)
        b13_sb = consts.tile([128, HC], dt.float32)
        nc.sync.dma_start(out=b13_sb[:], in_=b13[:])
        for dc in range(1, DC):
            nc.sync.dma_start(out=w13_sb[:, dc, :], in_=w13[:, dc, :])
        nc.sync.dma_start(out=w2_sb[:], in_=w2[:])

        def logits_ntile(nt, a_ps):
            """One 512-instance tile of the gated-attention logit path,
            ending in exp(logits) for its 4 chunks -> e_sb [128, 16]."""
            if True:
                n0 = nt * NTILE
                xt = xtp.tile([128, DC, NTILE], dt.bfloat16, tag="xt")
                if nt == 0:  # split: first rhs chunk lands sooner
                    for dc in range(DC):
                        nc.sync.dma_start(out=xt[:, dc, :], in_=xT_v[:, dc, n0:n0 + NTILE])
                else:
                    nc.sync.dma_start(out=xt[:], in_=xT_v[:, :, n0:n0 + NTILE])
                ap_tiles = []
                for pair in range(2):  # h-chunks: tanh-branch, sigmoid-branch
                    h1 = psH.tile([128, NTILE], dt.float32, tag="h")
                    for dc in range(DC):
                        nc.tensor.matmul(
                            h1[:],
                            w13_sb[:, dc, pair * 128:(pair + 1) * 128],
                            xt[:, dc, :],
                            start=(dc == 0),
                            stop=(dc == DC - 1),
                        )
                    t = tsp.tile([128, NTILE], dt.bfloat16, tag="ts")
                    nc.scalar.activation(
                        t[:], h1[:], AF.Tanh, bias=b13_sb[:, pair:pair + 1], scale=1.0
                    )
                    h3 = psH.tile([128, NTILE], dt.float32, tag="h")
                    for dc in range(DC):
                        nc.tensor.matmul(
                            h3[:],
                            w13_sb[:, dc, (pair + 2) * 128:(pair + 3) * 128],
                            xt[:, dc, :],
                            start=(dc == 0),
                            stop=(dc == DC - 1),
                        )
                    s = tsp.tile([128, NTILE], dt.bfloat16, tag="ts")
                    nc.scalar.activation(
                        s[:], h3[:], AF.Tanh, bias=b13_sb[:, pair + 2:pair + 3], scale=0.5
                    )
                    # gated = t*sigmoid = 0.5*(t*s + t); the 0.5 lives in w2
                    apt = app.tile([128, NTILE], dt.bfloat16, tag="ap")
                    nc.vector.tensor_mul(out=apt[:], in0=t[:], in1=s[:])
                    nc.vector.tensor_add(out=apt[:], in0=apt[:], in1=t[:])
                    ap_tiles.append(apt)
                ntl = nt % NTB
                for sub in range(NTILE // 128):
                    c = ntl * (NTILE // 128) + sub
                    for pair in range(2):
                        nc.tensor.matmul(
                            a_ps[:, K * c:K * (c + 1)],
                            ap_tiles[pair][:, sub * 128:(sub + 1) * 128],
                            w2_sb[:, pair, :],
                            start=(pair == 0),
                            stop=(pair == 1),
                        )
                # exp is per-element (no max-subtraction), so this n-tile's
                # weights are ready as soon as its own logits are.
                e_nt = epool.tile([128, K * (NTILE // 128)], dt.bfloat16, tag="e")
                nc.scalar.activation(
                    e_nt[:], a_ps[:, K * 4 * ntl:K * 4 * (ntl + 1)], AF.Exp
                )
                return e_nt

        def wsum_group(bag, grp, e_nt, u_ps, z_ps):
            """4 x-chunks of one bag's exp-weighted sum + normalizer."""
            g0 = bag * NCH + grp * 4
            xat = xap.tile([128, 4, 513], dt.bfloat16, tag="xa")
            nc.sync.dma_start(out=xat[:], in_=xa_v[:, g0:g0 + 4, :])
            for j in range(4):
                c = grp * 4 + j
                lhs = e_nt[:, K * j:K * (j + 1)]
                nc.tensor.matmul(
                    u_ps[:], lhs, xat[:, j, 0:D],
                    start=(c == 0), stop=(c == NCH - 1),
                )
                nc.tensor.matmul(
                    z_ps[:], lhs, xat[:, j, D:513],
                    start=(c == 0), stop=(c == NCH - 1),
                )

        def wsum_finish(bag, u_ps, z_ps):
            r_sb = rpool.tile([K, 1], dt.float32, tag="r")
            nc.vector.reciprocal(out=r_sb[:], in_=z_ps[:])
            o_sb = opool.tile([K, D], dt.float32, tag="o")
            nc.vector.tensor_scalar_mul(out=o_sb[:], in0=u_ps[:], scalar1=r_sb[:])
            nc.sync.dma_start(out=out[bag], in_=o_sb[:])

        # Software pipeline with a one-n-tile skew: the weighted-sum group of
        # n-tile i runs behind the logits of n-tile i+1, so the PE always has
        # dense matmul work ahead of any x-chunk DMA wait and the HAM clock
        # stays warm until the final group.
        bag_state = {}
        prev = None  # (bag, grp, e_nt)
        for bag in range(BPC):
            a_ps = psA.tile([128, K * NCH], dt.float32, tag="a")
            u_ps = psU.tile([K, D], dt.float32, tag="u")
            z_ps = psZ.tile([K, 1], dt.float32, tag="z")
            bag_state[bag] = (u_ps, z_ps)
            for ntl in range(NTB):
                e_nt = logits_ntile(bag * NTB + ntl, a_ps)
                if prev is not None:
                    pb, pg, pe = prev
                    pu, pz = bag_state[pb]
                    wsum_group(pb, pg, pe, pu, pz)
                    if pg == NTB - 1:
                        wsum_finish(pb, pu, pz)
                prev = (bag, ntl, e_nt)
        pb, pg, pe = prev
        pu, pz = bag_state[pb]
        wsum_group(pb, pg, pe, pu, pz)
        wsum_finish(pb, pu, pz)

    nc.compile()
    return nc


def get_nc():
    if "nc" not in _CACHE:
        _CACHE["nc"] = _build_nc()
    return _CACHE["nc"]


def make_in_maps(x, W1, b1, W3, b3, W2, b2):
    x = np.asarray(x, dtype=np.float32)
    W1 = np.asarray(W1, dtype=np.float32)
    W3 = np.asarray(W3, dtype=np.float32)
    W2 = np.asarray(W2, dtype=np.float32)
    b1 = np.asarray(b1, dtype=np.float32)
    b3 = np.asarray(b3, dtype=np.float32)

    # [W1 | W3] with lhsT layout [p, dc, h']: element = W13[dc*128+p, h']
    w13 = np.concatenate([W1, W3], axis=1)          # [512, 512]
    w13_t = np.ascontiguousarray(
        w13.reshape(DC, 128, 2 * H).transpose(1, 0, 2)
    ).astype(_BF16)
    # 0.5 * W2 with layout [p, hc, k]
    w2_t = np.ascontiguousarray(
        (0.5 * W2).reshape(2, 128, K).transpose(1, 0, 2)
    ).astype(_BF16)
    # biases [p, j]: j in {0,1} -> b1 chunks, {2,3} -> 0.5*b3 chunks
    b13 = np.concatenate([b1, 0.5 * b3]).reshape(HC, 128).T
    b13 = np.ascontiguousarray(b13, dtype=np.float32)

    in_maps = []
    for c in range(NCORES):
        xc = x[c * R:(c + 1) * R]                   # [8192, 512] fp32
        xT_np = np.ascontiguousarray(xc.T).astype(_BF16)
        xa_np = np.empty((R, 513), dtype=_BF16)
        xa_np[:, :D] = xc.astype(_BF16)
        xa_np[:, D] = _BF16(1.0)
        in_maps.append(
            {"xT": xT_np, "xa": xa_np, "w13": w13_t, "w2": w2_t, "b13": b13}
        )
    return in_maps


def kernel(x, W1, b1, W3, b3, W2, b2, bag_lengths):
    from concourse.bass_utils import run_bass_kernel_spmd

    nc = get_nc()
    in_maps = make_in_maps(x, W1, b1, W3, b3, W2, b2)
    res = run_bass_kernel_spmd(nc, in_maps, list(range(NCORES)))
    out = np.empty((B, K * D), dtype=np.float32)
    for c in range(NCORES):
        out[c * BPC:(c + 1) * BPC] = res.results[c]["out"].reshape(BPC, K * D)
    return out


# revision 11
# speedup vs baseline: 1.2947x; 1.2834x over previous
"""MIL gated-attention pooling kernel for Trainium2 (8 NeuronCores, SPMD).

Problem (per reference):
    A_pre = tanh(x@W1 + b1) * sigmoid(x@W3 + b3)      # [N, H]
    A     = A_pre @ W2 + b2                           # [N, K]
    P     = softmax over instances per (bag, head)    # [B, K, L]
    out   = einsum('bkl,bld->bkd', P, x) -> [B, K*D]

Shapes hardcoded: B=32 bags, L=2048 instances/bag, D=512, H=256, K=4.
Sharding: data-parallel over bags, 4 bags (8192 rows) per core, weights
replicated. No cross-core communication.

Device algorithm per core (all matmuls bf16 inputs, fp32 PSUM accum):
  - logits path consumes a host-pretransposed xT [D, 8192] so the
    contraction dim (d) sits on SBUF partitions for the PE.
  - sigmoid(h) = 0.5*(1+tanh(h/2)); the 0.5 is folded into W2 on the
    host, so ACT only ever needs tanh+exp (one table set, no thrash).
  - b2 and the softmax max-subtraction are dropped: both cancel in
    softmax (b2 is constant along the softmax axis; logits are O(1)).
  - softmax denominator Z comes from a ones-column appended to x on the
    host, accumulated by the same PE weights as the weighted sum.
"""

import numpy as np
import ml_dtypes
from contextlib import ExitStack

B, L, D, H, K = 32, 2048, 512, 256, 4
NCORES = 8
BPC = B // NCORES       # bags per core = 4
R = BPC * L             # rows per core = 8192
NTILE = 512             # instance-tile (matmul free dim)
NT = R // NTILE         # n-tiles per core = 16
NTB = L // NTILE        # n-tiles per bag = 4
NCH = L // 128          # 128-row chunks per bag = 16
DC = D // 128           # contraction chunks = 4
HC = 2 * H // 128       # output-channel chunks of [W1|W3] = 4

_BF16 = ml_dtypes.bfloat16
_FP8 = ml_dtypes.float8_e4m3
FP8 = True  # fp8 DoubleRow for the big [W1|W3] matmuls
_CACHE = {}


def _build_nc():
    import concourse.bacc as bacc
    import concourse.tile as tile
    import concourse.mybir as mybir

    dt = mybir.dt
    AF = mybir.ActivationFunctionType

    nc = bacc.Bacc("TRN2", target_bir_lowering=False, debug=False)
    if FP8:
        # packed for DoubleRow: [p, 2*dc2+r, n] = xT[dc2*256 + r*128 + p, n]
        xT = nc.dram_tensor("xT", [128, DC, R], dt.float8e4, kind="ExternalInput").ap()
    else:
        xT = nc.dram_tensor("xT", [D, R], dt.bfloat16, kind="ExternalInput").ap()
    xa = nc.dram_tensor("xa", [R, 513], dt.bfloat16, kind="ExternalInput").ap()
    if FP8:
        # [p, dc2, r, h'] = 16*W13[dc2*256 + r*128 + p, h']
        w13 = nc.dram_tensor("w13", [128, 2, 2, 2 * H], dt.float8e4, kind="ExternalInput").ap()
    else:
        w13 = nc.dram_tensor("w13", [128, DC, 2 * H], dt.bfloat16, kind="ExternalInput").ap()
    w2 = nc.dram_tensor("w2", [128, 2, K], dt.bfloat16, kind="ExternalInput").ap()
    b13 = nc.dram_tensor("b13", [128, HC], dt.float32, kind="ExternalInput").ap()
    out = nc.dram_tensor("out", [BPC, K, D], dt.float32, kind="ExternalOutput").ap()

    # transposed-DMA views: xT as [p, dc, n]; xa as [p, chunk, col]
    xT_v = xT if FP8 else xT.rearrange("(dc p) n -> p dc n", p=128)
    xa_v = xa.rearrange("(g p) f -> p g f", p=128)  # g = global 128-row chunk

    with tile.TileContext(nc) as tc, ExitStack() as ctx:
        consts = ctx.enter_context(tc.tile_pool(name="consts", bufs=1))
        xtp = ctx.enter_context(tc.tile_pool(name="xtp", bufs=4))
        tsp = ctx.enter_context(tc.tile_pool(name="tsp", bufs=8))
        app = ctx.enter_context(tc.tile_pool(name="app", bufs=6))
        epool = ctx.enter_context(tc.tile_pool(name="epool", bufs=4))
        xap = ctx.enter_context(tc.tile_pool(name="xap", bufs=5))
        opool = ctx.enter_context(tc.tile_pool(name="opool", bufs=2))
        rpool = ctx.enter_context(tc.tile_pool(name="rpool", bufs=2))
        import concourse.bass as bass

        psH = ctx.enter_context(tc.tile_pool(name="psH", bufs=4, space=bass.MemorySpace.PSUM))
        psA = ctx.enter_context(tc.tile_pool(name="psA", bufs=1, space=bass.MemorySpace.PSUM))
        psU = ctx.enter_context(tc.tile_pool(name="psU", bufs=2, space=bass.MemorySpace.PSUM))
        psZ = ctx.enter_context(tc.tile_pool(name="psZ", bufs=1, space=bass.MemorySpace.PSUM))

        # constants: w13 split per-dc so the first [128,128] weight block and
        # the first rhs chunk land ASAP after the DMA path warms up.
        if FP8:
            w13_sb = consts.tile([128, 2, 2, 2 * H], dt.float8e4)
            nc.sync.dma_start(out=w13_sb[:, 0], in_=w13[:, 0])
        else:
            w13_sb = consts.tile([128, DC, 2 * H], dt.bfloat16)
            nc.sync.dma_start(out=w13_sb[:, 0, :], in_=w13[:, 0, :])
        w2_sb = consts.tile([128, 2, K], dt.bfloat16)
        b13_sb = consts.tile([128, HC], dt.float32)
        nc.sync.dma_start(out=b13_sb[:], in_=b13[:])
        if FP8:
            nc.sync.dma_start(out=w13_sb[:, 1], in_=w13[:, 1])
        else:
            for dc in range(1, DC):
                nc.sync.dma_start(out=w13_sb[:, dc, :], in_=w13[:, dc, :])
        nc.sync.dma_start(out=w2_sb[:], in_=w2[:])

        def logits_ntile(nt, a_ps):
            """One 512-instance tile of the gated-attention logit path,
            ending in exp(logits) for its 4 chunks -> e_sb [128, 16]."""
            if True:
                n0 = nt * NTILE
                xt = xtp.tile([128, DC, NTILE], dt.float8e4 if FP8 else dt.bfloat16, tag="xt")
                if nt == 0:  # split: first rhs chunk lands sooner
                    for dc in range(DC):
                        nc.sync.dma_start(out=xt[:, dc, :], in_=xT_v[:, dc, n0:n0 + NTILE])
                else:
                    nc.sync.dma_start(out=xt[:], in_=xT_v[:, :, n0:n0 + NTILE])
                ap_tiles = []
                for pair in range(2):  # h-chunks: tanh-branch, sigmoid-branch
                    h1 = psH.tile([128, NTILE], dt.float32, tag="h")
                    if FP8:
                        for dc2 in range(2):
                            nc.tensor.matmul(
                                h1[:],
                                w13_sb[:, dc2, :, pair * 128:(pair + 1) * 128],
                                xt[:, 2 * dc2:2 * dc2 + 2, :],
                                start=(dc2 == 0), stop=(dc2 == 1),
                                perf_mode=mybir.MatmulPerfMode.DoubleRow,
                            )
                    else:
                        for dc in range(DC):
                            nc.tensor.matmul(
                                h1[:],
                                w13_sb[:, dc, pair * 128:(pair + 1) * 128],
                                xt[:, dc, :],
                                start=(dc == 0),
                                stop=(dc == DC - 1),
                            )
                    t = tsp.tile([128, NTILE], dt.bfloat16, tag="ts")
                    nc.scalar.activation(
                        t[:], h1[:], AF.Tanh, bias=b13_sb[:, pair:pair + 1],
                        scale=(1.0 / 16.0 if FP8 else 1.0),
                    )
                    h3 = psH.tile([128, NTILE], dt.float32, tag="h")
                    if FP8:
                        for dc2 in range(2):
                            nc.tensor.matmul(
                                h3[:],
                                w13_sb[:, dc2, :, (pair + 2) * 128:(pair + 3) * 128],
                                xt[:, 2 * dc2:2 * dc2 + 2, :],
                                start=(dc2 == 0), stop=(dc2 == 1),
                                perf_mode=mybir.MatmulPerfMode.DoubleRow,
                            )
                    else:
                        for dc in range(DC):
                            nc.tensor.matmul(
                                h3[:],
                                w13_sb[:, dc, (pair + 2) * 128:(pair + 3) * 128],
                                xt[:, dc, :],
                                start=(dc == 0),
                                stop=(dc == DC - 1),
                            )
                    s = tsp.tile([128, NTILE], dt.bfloat16, tag="ts")
                    nc.scalar.activation(
                        s[:], h3[:], AF.Tanh, bias=b13_sb[:, pair + 2:pair + 3],
                        scale=(0.5 / 16.0 if FP8 else 0.5),
                    )
                    # gated = t*sigmoid = 0.5*(t*s + t); the 0.5 lives in w2
                    apt = app.tile([128, NTILE], dt.bfloat16, tag="ap")
                    nc.vector.tensor_mul(out=apt[:], in0=t[:], in1=s[:])
                    nc.vector.tensor_add(out=apt[:], in0=apt[:], in1=t[:])
                    ap_tiles.append(apt)
                ntl = nt % NTB
                for sub in range(NTILE // 128):
                    c = ntl * (NTILE // 128) + sub
                    for pair in range(2):
                        nc.tensor.matmul(
                            a_ps[:, K * c:K * (c + 1)],
                            ap_tiles[pair][:, sub * 128:(sub + 1) * 128],
                            w2_sb[:, pair, :],
                            start=(pair == 0),
                            stop=(pair == 1),
                        )
                # exp is per-element (no max-subtraction), so this n-tile's
                # weights are ready as soon as its own logits are.
                e_nt = epool.tile([128, K * (NTILE // 128)], dt.bfloat16, tag="e")
                nc.scalar.activation(
                    e_nt[:], a_ps[:, K * 4 * ntl:K * 4 * (ntl + 1)], AF.Exp
                )
                return e_nt

        def wsum_group(bag, grp, e_nt, u_ps, z_ps):
            """4 x-chunks of one bag's exp-weighted sum + normalizer."""
            g0 = bag * NCH + grp * 4
            xat = xap.tile([128, 4, 513], dt.bfloat16, tag="xa")
            nc.sync.dma_start(out=xat[:], in_=xa_v[:, g0:g0 + 4, :])
            for j in range(4):
                c = grp * 4 + j
                lhs = e_nt[:, K * j:K * (j + 1)]
                nc.tensor.matmul(
                    u_ps[:], lhs, xat[:, j, 0:D],
                    start=(c == 0), stop=(c == NCH - 1),
                )
                nc.tensor.matmul(
                    z_ps[:], lhs, xat[:, j, D:513],
                    start=(c == 0), stop=(c == NCH - 1),
                )

        def wsum_finish(bag, u_ps, z_ps):
            r_sb = rpool.tile([K, 1], dt.float32, tag="r")
            nc.vector.reciprocal(out=r_sb[:], in_=z_ps[:])
            o_sb = opool.tile([K, D], dt.float32, tag="o")
            nc.vector.tensor_scalar_mul(out=o_sb[:], in0=u_ps[:], scalar1=r_sb[:])
            nc.sync.dma_start(out=out[bag], in_=o_sb[:])

        # Software pipeline with a one-n-tile skew: the weighted-sum group of
        # n-tile i runs behind the logits of n-tile i+1, so the PE always has
        # dense matmul work ahead of any x-chunk DMA wait and the HAM clock
        # stays warm until the final group.
        bag_state = {}
        prev = None  # (bag, grp, e_nt)
        for bag in range(BPC):
            a_ps = psA.tile([128, K * NCH], dt.float32, tag="a")
            u_ps = psU.tile([K, D], dt.float32, tag="u")
            z_ps = psZ.tile([K, 1], dt.float32, tag="z")
            bag_state[bag] = (u_ps, z_ps)
            for ntl in range(NTB):
                e_nt = logits_ntile(bag * NTB + ntl, a_ps)
                if prev is not None:
                    pb, pg, pe = prev
                    pu, pz = bag_state[pb]
                    wsum_group(pb, pg, pe, pu, pz)
                    if pg == NTB - 1:
                        wsum_finish(pb, pu, pz)
                prev = (bag, ntl, e_nt)
        pb, pg, pe = prev
        pu, pz = bag_state[pb]
        wsum_group(pb, pg, pe, pu, pz)
        wsum_finish(pb, pu, pz)

    nc.compile()
    return nc


def get_nc():
    if "nc" not in _CACHE:
        _CACHE["nc"] = _build_nc()
    return _CACHE["nc"]


def make_in_maps(x, W1, b1, W3, b3, W2, b2):
    x = np.asarray(x, dtype=np.float32)
    W1 = np.asarray(W1, dtype=np.float32)
    W3 = np.asarray(W3, dtype=np.float32)
    W2 = np.asarray(W2, dtype=np.float32)
    b1 = np.asarray(b1, dtype=np.float32)
    b3 = np.asarray(b3, dtype=np.float32)

    # [W1 | W3] with lhsT layout [p, dc, h']: element = W13[dc*128+p, h']
    w13 = np.concatenate([W1, W3], axis=1)          # [512, 512]
    if FP8:
        # [p, dc2, r, h'] = 16*W13[dc2*256 + r*128 + p, h']
        w13_t = np.ascontiguousarray(
            (16.0 * w13).reshape(2, 2, 128, 2 * H).transpose(2, 0, 1, 3)
        ).astype(_FP8)
    else:
        w13_t = np.ascontiguousarray(
            w13.reshape(DC, 128, 2 * H).transpose(1, 0, 2)
        ).astype(_BF16)
    # 0.5 * W2 with layout [p, hc, k]
    w2_t = np.ascontiguousarray(
        (0.5 * W2).reshape(2, 128, K).transpose(1, 0, 2)
    ).astype(_BF16)
    # biases [p, j]: j in {0,1} -> b1 chunks, {2,3} -> 0.5*b3 chunks
    b13 = np.concatenate([b1, 0.5 * b3]).reshape(HC, 128).T
    b13 = np.ascontiguousarray(b13, dtype=np.float32)

    in_maps = []
    for c in range(NCORES):
        xc = x[c * R:(c + 1) * R]                   # [8192, 512] fp32
        if FP8:
            # [p, 2*dc2+r, n] = xT[dc2*256 + r*128 + p, n]
            xT_np = np.ascontiguousarray(
                xc.T.reshape(2, 2, 128, R).transpose(2, 0, 1, 3).reshape(128, DC, R)
            ).astype(_FP8)
        else:
            xT_np = np.ascontiguousarray(xc.T).astype(_BF16)
        xa_np = np.empty((R, 513), dtype=_BF16)
        xa_np[:, :D] = xc.astype(_BF16)
        xa_np[:, D] = _BF16(1.0)
        in_maps.append(
            {"xT": xT_np, "xa": xa_np, "w13": w13_t, "w2": w2_t, "b13": b13}
        )
    return in_maps


def kernel(x, W1, b1, W3, b3, W2, b2, bag_lengths):
    from concourse.bass_utils import run_bass_kernel_spmd

    nc = get_nc()
    in_maps = make_in_maps(x, W1, b1, W3, b3, W2, b2)
    res = run_bass_kernel_spmd(nc, in_maps, list(range(NCORES)))
    out = np.empty((B, K * D), dtype=np.float32)
    for c in range(NCORES):
        out[c * BPC:(c + 1) * BPC] = res.results[c]["out"].reshape(BPC, K * D)
    return out
